# revision 17
# baseline (speedup 1.0000x reference)
"""Trainium2 Bass kernel for nn_BigGNN (16-iter cross-graph GAT message passing).

Strategy (8 NeuronCores, SPMD, node-parallel sharding):
  - core k owns x1 rows [128k,128k+128) and x2 rows [64k,64k+64).
  - Dense formulation: sparse graphs -> dense masked attention with
    host-prebuilt transposed multiplicity masks (self loops on diagonal);
    cross-graph GNNs are full-bipartite (self loop via diagonal matmul).
  - exp(leaky_relu(as_j + ad_i)) == max(exp(as_j)exp(ad_i),
    exp(.2 as_j)exp(.2 ad_i)) -- rank-1 outer products; attention built in
    3-4 wide DVE ops per stage using step-0 broadcast APs.
  - Softmax denominator CANCELS: b == 0 so
    l2norm(relu(num/den)) == l2norm(relu(num)) (den > 0).
  - Row normalization deferred: epilogue computes rn = 1/||relu(num)||
    and the next projection's PSUM->SBUF copy applies scale=rn.
  - Exactly 2 AllGathers per iteration (projected z of owned rows).
  - All matmuls float32r (full PE rate; even-free-dim ISA restrictions).
  - The runtime here has ~40-70us per-instruction overhead, so the design
    minimizes instruction count (wide fused ops) over engine balance.
"""

import numpy as np

import concourse.bass as bass
import concourse.bacc as bacc
import concourse.mybir as mybir
import concourse.tile as tile
from concourse.bass_utils import run_bass_kernel_spmd

F32 = mybir.dt.float32
F32R = mybir.dt.float32r
AF = mybir.ActivationFunctionType
OP = mybir.AluOpType

NCORES = 8
N1, N2, D = 1024, 512, 300
R1, R2 = N1 // NCORES, N2 // NCORES  # 128, 64
DC = [(0, 128), (128, 128), (256, 44)]
ZW = 304  # z cols: 0:300 z, 300 zero, 301 as, 302 ad, 303 pad
B = R1 + R2  # 192 rows per core in the collective


def build_nc(niter=16, fake_ag=False):
    nc = bacc.Bacc("TRN2", target_bir_lowering=False, debug=False,
                   num_devices=NCORES)
    # register the sqrt-bias constant (pre-TileContext, like the builtins)
    _c = nc.alloc_sbuf_tensor("const-f32-1em24", [128, 1], F32)
    nc.gpsimd.memset(_c.ap(), 1e-24)
    nc.const_aps.aps[(F32, 1e-24)] = _c.ap()
    nc.all_engine_barrier()

    x1_in = nc.dram_tensor("x1_in", [R1, D], F32, kind="ExternalInput")
    x2_in = nc.dram_tensor("x2_in", [R2, D], F32, kind="ExternalInput")
    m1T_in = nc.dram_tensor("m1T", [N1, R1], F32, kind="ExternalInput")
    m2T_in = nc.dram_tensor("m2T", [N2, R2], F32, kind="ExternalInput")
    w_in = {nm: nc.dram_tensor(f"w{nm}", [D, ZW], F32, kind="ExternalInput")
            for nm in ("ts", "gs", "tc", "gc")}
    ident_in = nc.dram_tensor("ident", [128, 128], F32, kind="ExternalInput")
    ones_in = nc.dram_tensor("ones_row", [1, 128], F32, kind="ExternalInput")
    w1_in = nc.dram_tensor("mw1", [600, 600], F32, kind="ExternalInput")
    w2_in = nc.dram_tensor("mw2", [600, D], F32, kind="ExternalInput")
    w3_in = nc.dram_tensor("mw3", [D, 2], F32, kind="ExternalInput")
    x1_out = nc.dram_tensor("x1_out", [R1, D], F32, kind="ExternalOutput")
    x2_out = nc.dram_tensor("x2_out", [R2, D], F32, kind="ExternalOutput")
    sc_out = nc.dram_tensor("sc_out", [1, 1], F32, kind="ExternalOutput")

    with tile.TileContext(nc) as tc:
        with (
            tc.tile_pool(name="const", bufs=1) as const,
            tc.tile_pool(name="state", bufs=2) as state,
            tc.tile_pool(name="gath", bufs=2) as gath,
            tc.tile_pool(name="work", bufs=2) as work,
            tc.tile_pool(name="aggp", bufs=2, space="PSUM") as aggp,
            tc.tile_pool(name="zp", bufs=2, space="PSUM") as zp,
            tc.tile_pool(name="up", bufs=2, space="PSUM") as up,
            tc.tile_pool(name="tp", bufs=2, space="PSUM") as tp,
            tc.tile_pool(name="dram", bufs=1, space="DRAM") as dram,
        ):
            # ---------------- constants ----------------
            ident = const.tile([128, 128], F32R, name="ident_sb")
            nc.sync.dma_start(ident[:], ident_in[:].bitcast(F32R))
            ones_row = const.tile([1, 128], F32R, name="ones_sb")
            nc.sync.dma_start(ones_row[:], ones_in[:].bitcast(F32R))
            wext = {}
            for nm, t in w_in.items():
                wsb = const.tile([128, 3, ZW], F32R, name=f"w_{nm}_sb")
                for c, (off, w) in enumerate(DC):
                    nc.sync.dma_start(wsb[:w, c, :],
                                      t[off:off + w, :].bitcast(F32R))
                wext[nm] = wsb
            m1all = const.tile([128, 8, R1], F32, name="m1_sb")
            nc.sync.dma_start(
                m1all[:], m1T_in[:].rearrange("(r p) d -> p r d", p=128))
            m2all = const.tile([64, 8, R2], F32, name="m2_sb")
            nc.sync.dma_start(
                m2all[:], m2T_in[:].rearrange("(r p) d -> p r d", p=64))

            rg = [list(range(NCORES))]

            # ---------------- helpers ----------------
            def allgather(tag, z1t, z2t):
                cin = dram.tile([B, ZW], F32R, name=f"cin_{tag}",
                                tag=f"cin_{tag}")
                gout = dram.tile([NCORES * B, ZW], F32R, name=f"gout_{tag}",
                                 tag=f"gout_{tag}",
                                 addr_space="Local" if fake_ag else "Shared")
                nc.sync.dma_start(cin[0:R1, :], z1t[:])
                nc.sync.dma_start(cin[R1:B, :], z2t[:])
                if fake_ag:
                    for r in range(NCORES):
                        nc.sync.dma_start(gout[B * r:B * (r + 1), :], cin[:])
                else:
                    nc.gpsimd.collective_compute(
                        "AllGather", OP.bypass, replica_groups=rg,
                        ins=[cin.opt()], outs=[gout.opt()])
                return gout

            def gather_in(gout, tag):
                z1g = gath.tile([128, 8, ZW], F32R, name=f"z1g_{tag}",
                                tag="z1g")
                z2g = gath.tile([64, 8, ZW], F32R, name=f"z2g_{tag}",
                                tag="z2g")
                g3 = gout[:].rearrange("(r q) f -> q r f", q=B)
                nc.sync.dma_start(z1g[:], g3[0:R1])
                nc.sync.dma_start(z2g[:], g3[R1:B])
                return z1g, z2g

            def transpose_x(xr, p, tag):
                """xr [p, 300] F32R -> xT [128, 3, 128] F32R (chunk c holds
                rows off:off+w transposed; valid partitions :w)."""
                tps = tp.tile([128, 384], F32, name=f"tx_{tag}", tag="tp")
                for c, (off, w) in enumerate(DC):
                    nc.tensor.transpose(
                        tps[:w, c * 128:c * 128 + p].bitcast(F32R),
                        xr[:, off:off + w], ident[0:p, 0:p])
                xT = state.tile([128, 3, 128], F32R, name=f"xT_{tag}",
                                tag=f"xT{p}")
                nc.scalar.activation(
                    xT[:, 0:2, 0:p],
                    tps[:, 0:256].rearrange("a (c f) -> a c f", c=2)[:, :, 0:p],
                    AF.Copy)
                nc.scalar.activation(xT[0:44, 2, 0:p],
                                     tps[0:44, 256:256 + p], AF.Copy)
                return xT

            def project(xT, wsb, p, rn, tag, role):
                """z = (rn * x) @ wext -> [p, ZW] F32R."""
                zps = zp.tile([p, ZW], F32, name=f"zp_{tag}", tag="zp")
                for c, (off, w) in enumerate(DC):
                    nc.tensor.matmul(zps[:], xT[:w, c, 0:p], wsb[:w, c, :],
                                     start=(c == 0), stop=(c == 2))
                z = state.tile([p, ZW], F32R, name=f"z_{tag}",
                               tag=f"zz_{role}")
                if rn is None:
                    nc.scalar.activation(z[:], zps[:], AF.Copy)
                else:
                    nc.scalar.activation(z[:], zps[:], AF.Copy, scale=rn[:])
                return z

            def build_U(z_dst, dstw, srcp, tag):
                """PSUM [srcp, 2*dstw]: cols 0:dstw = exp(ad_i) broadcast
                down partitions, dstw: = exp(.2 ad_i)."""
                tps = tp.tile([128, 384], F32, name=f"adT_{tag}", tag="tp")
                nc.tensor.transpose(tps[0:1, 0:dstw].bitcast(F32R),
                                    z_dst[:, 302:303], ident[0:dstw, 0:dstw])
                uu = work.tile([1, 2 * dstw], F32R, name=f"uu_{tag}",
                               tag=f"uu{dstw}")
                nc.scalar.activation(uu[:, 0:dstw], tps[0:1, 0:dstw], AF.Exp)
                nc.scalar.activation(uu[:, dstw:2 * dstw], tps[0:1, 0:dstw],
                                     AF.Exp, scale=0.2)
                ups = up.tile([128, 256], F32, name=f"U_{tag}", tag="up")
                nc.tensor.matmul(ups[0:srcp, 0:2 * dstw],
                                 ones_row[0:1, 0:srcp], uu[:],
                                 start=True, stop=True)
                return ups

            def build_PT(zg, ups, masks, srcp, dstw, tag):
                """PT [srcp, 8, dstw] F32R: (masked) rank-1 attention."""
                v = work.tile([srcp, 8], F32, name=f"v_{tag}", tag=f"v{srcp}")
                vh = work.tile([srcp, 8], F32, name=f"vh_{tag}",
                               tag=f"vh{srcp}")
                asrc = zg[:, :, 301].bitcast(F32)
                nc.scalar.activation(v[:], asrc, AF.Exp)
                nc.scalar.activation(vh[:], asrc, AF.Exp, scale=0.2)

                def bc_u(col0):
                    return ups[0:srcp, col0:col0 + dstw].rearrange(
                        "p (c f) -> p c f", c=1).broadcast_to(
                        [srcp, 8, dstw])

                def bc_v(vt):
                    return vt[:].rearrange("p (c f) -> p c f", f=1
                                           ).broadcast_to([srcp, 8, dstw])

                t1 = work.tile([srcp, 8, dstw], F32, name=f"t1_{tag}",
                               tag=f"t1{srcp}")
                nc.vector.tensor_tensor(out=t1[:], in0=bc_u(0), in1=bc_v(v),
                                        op=OP.mult)
                t2 = work.tile([srcp, 8, dstw], F32, name=f"t2_{tag}",
                               tag=f"t2{srcp}")
                nc.vector.tensor_tensor(out=t2[:], in0=bc_u(dstw),
                                        in1=bc_v(vh), op=OP.mult)
                pt = work.tile([srcp, 8, dstw], F32R, name=f"pt_{tag}",
                               tag=f"pt{srcp}")
                if masks is None:
                    nc.vector.tensor_tensor(out=pt[:], in0=t1[:], in1=t2[:],
                                            op=OP.max)
                else:
                    nc.vector.tensor_tensor(out=t1[:], in0=t1[:], in1=t2[:],
                                            op=OP.max)
                    nc.vector.tensor_tensor(out=pt[:], in0=t1[:],
                                            in1=masks, op=OP.mult)
                return pt

            def epilogue(ps, p, tag, materialize=False):
                """psum [p, >=300] -> xr [p,300] F32R (unnormalized),
                rn [p,1] ~= 1/max(||xr||, 1e-12)."""
                xr = state.tile([p, D], F32R, name=f"xr_{tag}",
                                tag=f"xr{tag[:2]}")
                nc.scalar.activation(xr[:], ps[:, 0:D], AF.Relu)
                sq = work.tile([p, D], F32, name=f"sq_{tag}", tag="sq")
                ssq = work.tile([p, 1], F32, name=f"ssq_{tag}", tag="ssq")
                nc.scalar.activation(sq[:], xr[:].bitcast(F32), AF.Square,
                                     accum_out=ssq[:])
                nrm = work.tile([p, 1], F32, name=f"nrm_{tag}", tag="nrm")
                nc.scalar.activation(nrm[:], ssq[:], AF.Sqrt, bias=1e-24)
                rn = state.tile([p, 1], F32, name=f"rn_{tag}",
                                tag=f"rn{tag[:2]}")
                nc.vector.reciprocal(rn[:], nrm[:])
                if not materialize:
                    return xr, rn
                xn = state.tile([p, D], F32R, name=f"xn_{tag}", tag=f"xn{p}")
                nc.scalar.activation(xn[:], xr[:].bitcast(F32), AF.Copy,
                                     scale=rn[:])
                return xn, rn

            def agg_stage(z_own, zg, masks, srcp, dstw, tag, self_loop):
                ups = build_U(z_own, dstw, srcp, tag)
                pt = build_PT(zg, ups, masks, srcp, dstw, tag)
                ps = aggp.tile([dstw, D], F32, name=f"agg_{tag}", tag="aggp")
                for r in range(NCORES):
                    nc.tensor.matmul(ps[:], pt[:, r, :], zg[:, r, 0:D],
                                     start=(r == 0),
                                     stop=(r == NCORES - 1 and not self_loop))
                if self_loop:
                    tse = work.tile([dstw, 1], F32, name=f"ts_{tag}",
                                    tag="tse")
                    nc.vector.tensor_tensor(
                        out=tse[:], in0=z_own[:, 301:302].bitcast(F32),
                        in1=z_own[:, 302:303].bitcast(F32), op=OP.add)
                    pe1 = work.tile([dstw, 1], F32, name=f"pa_{tag}",
                                    tag="pe1")
                    nc.scalar.activation(pe1[:], tse[:], AF.Exp)
                    pe2 = work.tile([dstw, 1], F32, name=f"pb_{tag}",
                                    tag="pe2")
                    nc.scalar.activation(pe2[:], tse[:], AF.Exp, scale=0.2)
                    psf = work.tile([dstw, 1], F32, name=f"pf_{tag}",
                                    tag="psf")
                    nc.vector.tensor_tensor(out=psf[:], in0=pe1[:],
                                            in1=pe2[:], op=OP.max)
                    diag = work.tile([dstw, dstw], F32R, name=f"dg_{tag}",
                                     tag=f"dg{dstw}")
                    nc.vector.tensor_scalar_mul(
                        diag[:], ident[0:dstw, 0:dstw].bitcast(F32), psf[:])
                    nc.tensor.matmul(ps[:], diag[:], z_own[:, 0:D],
                                     start=False, stop=True)
                return ps

            # ---------------- entry: load + norm + project ----------------
            xt1 = state.tile([R1, D], F32, name="xt1_raw", tag="xraw1")
            nc.sync.dma_start(xt1[:], x1_in[:])
            xt2 = state.tile([R2, D], F32, name="xt2_raw", tag="xraw2")
            nc.sync.dma_start(xt2[:], x2_in[:])

            def entry_norm(xt, p, tag):
                sq = work.tile([p, D], F32, name=f"sq_{tag}", tag="sq")
                ssq = work.tile([p, 1], F32, name=f"ssq_{tag}", tag="ssq")
                nc.scalar.activation(sq[:], xt[:], AF.Square,
                                     accum_out=ssq[:])
                nrm = work.tile([p, 1], F32, name=f"nrm_{tag}", tag="nrm")
                nc.scalar.activation(nrm[:], ssq[:], AF.Sqrt, bias=1e-24)
                rn = work.tile([p, 1], F32, name=f"rn_{tag}", tag="rni")
                nc.vector.reciprocal(rn[:], nrm[:])
                xn = state.tile([p, D], F32R, name=f"xn_{tag}", tag=f"xe{p}")
                nc.scalar.activation(xn[:], xt[:], AF.Copy, scale=rn[:])
                return xn

            x1n = entry_norm(xt1, R1, "e1")
            x2n = entry_norm(xt2, R2, "e2")
            xT1 = transpose_x(x1n, R1, "i1")
            xT2 = transpose_x(x2n, R2, "i2")
            z1 = project(xT1, wext["ts"], R1, None, "z1i", "z1")
            z2 = project(xT2, wext["gs"], R2, None, "z2i", "z2")
            gout_g = allgather("gi", z1, z2)

            for it in range(niter):
                s = str(it)
                # -------- round G --------
                z1g, z2g = gather_in(gout_g, f"g{s}")
                ps1 = agg_stage(z1, z1g, m1all[:], 128, R1, f"a1{s}", False)
                xr1, rn1 = epilogue(ps1, R1, f"e1{s}")
                ps2 = agg_stage(z2, z2g, m2all[:], 64, R2, f"a2{s}", False)
                xr2, rn2 = epilogue(ps2, R2, f"e2{s}")

                # -------- round C prep --------
                xT1 = transpose_x(xr1, R1, f"p1{s}")
                xT2 = transpose_x(xr2, R2, f"p2{s}")
                zc1o = project(xT1, wext["tc"], R1, rn1, f"c1o{s}", "c1o")
                zc2s = project(xT1, wext["gc"], R1, rn1, f"c2s{s}", "c2s")
                zc2o = project(xT2, wext["gc"], R2, rn2, f"c2o{s}", "c2o")
                zc1s = project(xT2, wext["tc"], R2, rn2, f"c1s{s}", "c1s")
                gout_c = allgather(f"c{s}", zc2s, zc1s)

                # -------- round C --------
                z1gc, z2gc = gather_in(gout_c, f"c{s}")
                pc1 = agg_stage(zc1o, z2gc, None, 64, R1, f"c1{s}", True)
                pc2 = agg_stage(zc2o, z1gc, None, 128, R2, f"c2{s}", True)
                last = it == niter - 1
                xq1, rq1 = epilogue(pc1, R1, f"f1{s}", materialize=last)
                xq2, rq2 = epilogue(pc2, R2, f"f2{s}", materialize=last)

                if not last:
                    # -------- round G prep (next iteration) --------
                    xT1 = transpose_x(xq1, R1, f"n1{s}")
                    xT2 = transpose_x(xq2, R2, f"n2{s}")
                    z1 = project(xT1, wext["ts"], R1, rq1, f"z1{s}", "z1")
                    z2 = project(xT2, wext["gs"], R2, rq2, f"z2{s}", "z2")
                    gout_g = allgather(f"g{s}", z1, z2)

            # ---------------- outputs + MLP ----------------
            nc.sync.dma_start(x1_out[:], xq1[:].bitcast(F32))
            nc.sync.dma_start(x2_out[:], xq2[:].bitcast(F32))

            w1sb = const.tile([128, 6, 600], F32R, name="w1_sb")
            w2sb = const.tile([128, 5, D], F32R, name="w2_sb")
            w3sb = const.tile([128, 3, 2], F32R, name="w3_sb")
            h_chunks = [(xq1, 0, 128), (xq1, 128, 128), (xq1, 256, 44),
                        (xq2, 0, 128), (xq2, 128, 128), (xq2, 256, 44)]
            roff = 0
            for j, (_, off, w) in enumerate(h_chunks):
                nc.sync.dma_start(w1sb[:w, j, :],
                                  w1_in[roff:roff + w, :].bitcast(F32R))
                roff += w
            for j in range(5):
                w = 128 if j < 4 else 88
                nc.sync.dma_start(w2sb[:w, j, :],
                                  w2_in[128 * j:128 * j + w, :].bitcast(F32R))
            for j, (off, w) in enumerate(DC):
                nc.sync.dma_start(w3sb[:w, j, :],
                                  w3_in[off:off + w, :].bitcast(F32R))

            def col_chunks(chunks, tag):
                ct = state.tile([128, len(chunks), 1], F32R,
                                name=f"ct_{tag}", tag=f"ct_{tag}")
                for j, (src, off, w) in enumerate(chunks):
                    tps = tp.tile([128, 384], F32, name=f"mt_{tag}{j}",
                                  tag="tp")
                    nc.tensor.transpose(tps[:w, 0:1],
                                        src[0:1, off:off + w].bitcast(F32),
                                        ident[0:1, 0:1].bitcast(F32))
                    nc.scalar.activation(ct[:w, j, :], tps[:w, 0:1], AF.Copy)
                return ct

            hT = col_chunks(h_chunks, "h")
            h1a = aggp.tile([1, D], F32, name="h1a", tag="aggp")
            h1b = aggp.tile([1, D], F32, name="h1b", tag="aggp")
            for j, (_, off, w) in enumerate(h_chunks):
                nc.tensor.matmul(h1a[:], hT[:w, j, :], w1sb[:w, j, 0:D],
                                 start=(j == 0), stop=(j == 5))
            for j, (_, off, w) in enumerate(h_chunks):
                nc.tensor.matmul(h1b[:], hT[:w, j, :], w1sb[:w, j, D:600],
                                 start=(j == 0), stop=(j == 5))
            h1 = state.tile([1, 600], F32R, name="h1_sb", tag="h1")
            nc.scalar.activation(h1[:, 0:D], h1a[:], AF.Relu)
            nc.scalar.activation(h1[:, D:600], h1b[:], AF.Relu)

            h1_chunks = [(h1, 128 * j, 128 if j < 4 else 88)
                         for j in range(5)]
            h1T = col_chunks(h1_chunks, "h1")
            h2p = aggp.tile([1, D], F32, name="h2p", tag="aggp")
            for j, (_, off, w) in enumerate(h1_chunks):
                nc.tensor.matmul(h2p[:], h1T[:w, j, :], w2sb[:w, j, :],
                                 start=(j == 0), stop=(j == 4))
            h2 = state.tile([1, D], F32R, name="h2_sb", tag="h2")
            nc.scalar.activation(h2[:], h2p[:], AF.Relu)

            h2_chunks = [(h2, off, w) for (off, w) in DC]
            h2T = col_chunks(h2_chunks, "h2")
            h3p = aggp.tile([1, 2], F32, name="h3p", tag="aggp")
            for j, (_, off, w) in enumerate(h2_chunks):
                nc.tensor.matmul(h3p[:], h2T[:w, j, :], w3sb[:w, j, :],
                                 start=(j == 0), stop=(j == 2))
            sc = state.tile([1, 1], F32, name="sc_sb", tag="sc")
            nc.scalar.activation(sc[:], h3p[:, 0:1], AF.Sigmoid)
            nc.sync.dma_start(sc_out[:], sc[:])

    nc.compile()
    return nc


# ---------------- host side ----------------

_NC_CACHE = {}


def _get_nc(niter=16):
    if niter not in _NC_CACHE:
        _NC_CACHE[niter] = build_nc(niter)
    return _NC_CACHE[niter]


def _build_maskT(src, dst, n):
    A = np.zeros((n, n), np.float32)
    np.add.at(A, (np.asarray(src, np.int64), np.asarray(dst, np.int64)), 1.0)
    A[np.arange(n), np.arange(n)] += 1.0
    return A


def _build_wext(W, a_s, a_d):
    W = np.asarray(W, np.float32)
    return np.concatenate(
        [W, np.zeros((D, 1), np.float32),
         (W @ np.asarray(a_s, np.float32))[:, None],
         (W @ np.asarray(a_d, np.float32))[:, None],
         np.zeros((D, 1), np.float32)], axis=1).astype(np.float32)


def make_in_maps(inputs):
    x1 = np.ascontiguousarray(np.asarray(inputs["x_1"], np.float32))
    x2 = np.ascontiguousarray(np.asarray(inputs["x_2"], np.float32))
    e1 = np.asarray(inputs["edge_index_1"])
    e2 = np.asarray(inputs["edge_index_2"])
    m1T = _build_maskT(e1[0], e1[1], N1)
    m2T = _build_maskT(e2[0], e2[1], N2)
    wx = {nm: _build_wext(inputs[f"W_{nm}"], inputs[f"as_{nm}"],
                          inputs[f"ad_{nm}"])
          for nm in ("ts", "gs", "tc", "gc")}
    ident = np.eye(128, dtype=np.float32)
    ones = np.ones((1, 128), np.float32)
    w1 = np.ascontiguousarray(np.asarray(inputs["mlp_W1"], np.float32))
    w2 = np.ascontiguousarray(np.asarray(inputs["mlp_W2"], np.float32))
    w3 = np.concatenate([np.asarray(inputs["mlp_W3"], np.float32),
                         np.zeros((D, 1), np.float32)], axis=1)
    in_maps = []
    for k in range(NCORES):
        m = {
            "x1_in": x1[R1 * k:R1 * (k + 1)],
            "x2_in": x2[R2 * k:R2 * (k + 1)],
            "m1T": np.ascontiguousarray(m1T[:, R1 * k:R1 * (k + 1)]),
            "m2T": np.ascontiguousarray(m2T[:, R2 * k:R2 * (k + 1)]),
            "ident": ident, "ones_row": ones,
            "mw1": w1, "mw2": w2, "mw3": w3,
        }
        for nm in ("ts", "gs", "tc", "gc"):
            m[f"w{nm}"] = wx[nm]
        in_maps.append(m)
    return in_maps


def kernel(**inputs):
    nc = _get_nc(16)
    in_maps = make_in_maps(inputs)
    res = run_bass_kernel_spmd(nc, in_maps, list(range(NCORES)))
    x1 = np.concatenate([res.results[k]["x1_out"] for k in range(NCORES)], 0)
    x2 = np.concatenate([res.results[k]["x2_out"] for k in range(NCORES)], 0)
    score = res.results[0]["sc_out"].reshape(1).astype(np.float32)
    return x1.astype(np.float32), x2.astype(np.float32), score
